# revision 43
# baseline (speedup 1.0000x reference)
"""Trainium2 Bass kernel: per-channel nearest-centroid (L1, K=4) VQ lookup.

Strategy (pure data parallel over 8 NeuronCores):
  - Host: shard melspecs [64,4096,80] along batch into 8 shards, transpose each
    shard to channel-major and view as [128, 20480] so that every 1024-column
    "band" of every partition row holds elements of a single channel.  All
    per-channel constants then become per-partition scalars (AP [128,1]).
  - Selection math: nearest centroid of a scalar among 4 sorted values is a
    3-step staircase.  Thresholds are computed on host by binary-searching the
    exact float32 crossover of the *reference* rule (argmin of fp32 |x-v| with
    first-index tie-break), so the device-side `x >= thr` decision is bit-exact
    equivalent to the reference selection for every representable x.
  - v2: the device emits the 2-bit RANK CODE, not the looked-up value, packed
    4 codes per byte.  That cuts per-core DMA from 21 MB (f32 in + f32 out) to
    11.1 MB (f32 in + u8/4 out), which is the roofline resource here.
      * c1  = (x >= t1)           tensor_scalar       (DVE)
      * c12 = (x >= t2) + c1      scalar_tensor_tensor(Pool mostly)
      * c3  = (x >= t3)           tensor_scalar       (DVE mostly)
    code = c12 + c3 in {0..3}, all masks bf16 (exact small ints).
  - PE packs 4 partition rows into one: for band j of each 4-band group, a
    constant block weight W_j[k, 32j + k//4] = 4^(k%4) turns two accumulating
    matmuls (c12, c3) into out[32j+p', col] = sum_i 4^i * code[4p'+i, col],
    i.e. base-4 digit packing into [32, 1024] per band, stacked 4 bands to a
    [128, 1024] PSUM group tile (values <= 255, exact in f32).
  - ACT copies PSUM -> SBUF uint8; DMA out is 1/16 the input traffic.
  - Host unpacks the 2-bit digits and applies the per-channel sorted-centroid
    LUT (exact; the device code is bit-identical to the reference assignment).
"""

import sys

for _p in ("/opt/trn_rl_repo",):
    if _p not in sys.path:
        sys.path.insert(0, _p)

import numpy as np

# Problem constants (hardcoded; kernel.py must be self-contained).
B, T, C, K = 64, 4096, 80, 4
NCORES = 8
BSH = B // NCORES          # batches per core
TOK = BSH * T              # tokens per core = 32768 (= elements per channel)
P = 128                    # SBUF partitions
ROW = TOK * C // P         # 20480 columns per partition
G = 1024                   # band width (columns); channel-pure per (row, band)
NB = ROW // G              # 20 bands
NG = NB // 4               # 5 groups of 4 bands packed per output byte-row
CHUNK = 512                # one matmul / PSUM-bank chunk

_PROG_CACHE = {}


# ---------------------------------------------------------------- host tables
def _key_of(u):
    # u: uint32 bits. negative floats (sign bit set) -> ~u ; positive -> u | 0x8000_0000
    return (~u) & 0xFFFFFFFF if (u & 0x80000000) else (u | 0x80000000)


def _bits_of_key(k):
    return (~k) & 0xFFFFFFFF if not (k & 0x80000000) else (k & 0x7FFFFFFF)


def _f32_from_key(k):
    return np.uint32(_bits_of_key(k)).view(np.float32)


def _rank_fn(cvals, pos_of_orig):
    cv = cvals.astype(np.float32)

    def rank(x):
        d = np.abs(np.float32(x) - cv)  # fp32, same as reference
        return pos_of_orig[int(np.argmin(d))]  # first-index tie-break

    return rank


def _exact_tables(centroids):
    """Per channel: sorted values and exact staircase thresholds.

    Returns thr [C,3], sv [C,K] (float32) such that
    reference_pick(x, channel c) == sv[c, (x>=thr[c,0])+(x>=thr[c,1])+(x>=thr[c,2])]
    for every float32 x.
    """
    cent = np.asarray(centroids, dtype=np.float32)
    thr = np.empty((C, 3), np.float32)
    sv_all = np.empty((C, K), np.float32)
    for c in range(C):
        cv = cent[c]
        order = np.argsort(cv, kind="stable")
        sv = cv[order]                       # sorted values
        sv_all[c] = sv
        pos_of_orig = np.empty(K, np.int64)
        pos_of_orig[order] = np.arange(K)
        rank = _rank_fn(cv, pos_of_orig)
        for j in range(3):
            lo = _key_of(int(np.float32(sv[j]).view(np.uint32)))
            hi = _key_of(int(np.float32(sv[j + 1]).view(np.uint32)))
            assert rank(_f32_from_key(lo)) <= j and rank(_f32_from_key(hi)) >= j + 1
            while hi - lo > 1:
                mid = (hi + lo) // 2
                if rank(_f32_from_key(mid)) >= j + 1:
                    hi = mid
                else:
                    lo = mid
            thr[c, j] = _f32_from_key(hi)    # smallest f32 picking rank >= j+1
    return thr, sv_all


def _band_channel(p, k):
    """Channel owning band k of partition row p (channel-major flat layout)."""
    return (p * ROW + k * G) // TOK


def _make_tab(thr):
    """Per-(partition, band) threshold scalars (3 blocks of NB columns) plus
    the per-partition digit scale 4^(p%4) used to weight the masks."""
    tab = np.empty((P, 3 * NB + 1), np.float32)
    for p in range(P):
        for k in range(NB):
            c = _band_channel(p, k)
            for i in range(3):
                tab[p, i * NB + k] = thr[c, i]
        tab[p, 3 * NB] = float(4 ** (p % 4))
    return tab


def _w_digit(j):
    """Base-4 digit-packing weight W_j [128,128]: maps code rows 4p'..4p'+3 of
    digit-position j to packed row 32j+p' with weights 4^(row%4)."""
    w = np.zeros((P, P), np.float32)
    for kk in range(P):
        w[kk, 32 * j + kk // 4] = float(4 ** (kk % 4))
    return w


def _make_wts():
    """Six DoubleRow weight pairs packed as [128, 12, 128] fp8e4 (exact
    powers of 4).

    Pair j   (j=0..3): (W_j, W_j)   — consumes (c1, c3) of the group's
                                      band at digit position j.
    Pair 4 / 5: (W_0, W_1) / (W_2, W_3) — consumes (c2, c2) of digit
                                      positions (0,1) / (2,3).
    """
    import ml_dtypes

    pairs = [(j, j) for j in range(4)] + [(0, 1), (2, 3)]
    w = np.stack([_w_digit(j) for a, b in pairs for j in (a, b)], axis=1)
    return w.astype(ml_dtypes.float8_e4m3)  # [128, 12, 128]


# ---------------------------------------------------------------- device code
# Tail schedule (found by TimelineSim search): number of input/mask(c1,c3)
# pieces per band (1 piece = whole 1024-col band), per-piece c2 engine
# ('g'=Pool gpsimd, 'v'=DVE) with the c2 piece count given by the string
# length (independent of the input split), and the PSUM chunk width of the
# last group.
SPLITS = {19: 2}
C2ENG = {15: "v", 16: "v"}       # band -> engine string, one char per c2 piece
TAIL_CHUNK = 512


def _build_program(splits=None, c2eng=None, tail_chunk=None):
    import concourse.bacc as bacc
    import concourse.tile as tile
    from concourse import mybir

    splits = SPLITS if splits is None else splits
    c2eng = C2ENG if c2eng is None else c2eng
    tail_chunk = TAIL_CHUNK if tail_chunk is None else tail_chunk

    f32 = mybir.dt.float32
    fp8 = mybir.dt.float8e4
    u8 = mybir.dt.uint8
    alu = mybir.AluOpType
    dr = mybir.MatmulPerfMode.DoubleRow

    nc = bacc.Bacc("TRN2", target_bir_lowering=False, debug=False)
    x = nc.dram_tensor("x", [P, ROW], f32, kind="ExternalInput")
    tab = nc.dram_tensor("tab", [P, 3 * NB + 1], f32, kind="ExternalInput")
    wd = nc.dram_tensor("wts", [P, 12, P], fp8, kind="ExternalInput")
    y = nc.dram_tensor("y", [P, NG * G], u8, kind="ExternalOutput")

    with tile.TileContext(nc) as tc:
        with (
            tc.tile_pool(name="const", bufs=1) as cpool,
            tc.tile_pool(name="xin", bufs=8) as xpool,
            tc.tile_pool(name="pab", bufs=5) as pabpool,
            tc.tile_pool(name="pc", bufs=3) as pcpool,
            tc.tile_pool(name="acc", bufs=3, space="PSUM") as ppool,
            tc.tile_pool(name="out", bufs=5) as opool,
        ):
            # consts go on the ACT HWDGE queue so SP's first issue is x band 0
            tabt = cpool.tile([P, 3 * NB + 1], f32)
            nc.scalar.dma_start(out=tabt[:], in_=tab[:])
            wtile = cpool.tile([P, 12, P], fp8)
            nc.scalar.dma_start(out=wtile[:], in_=wd[:])

            def col(i, k):  # threshold i (0..2) scalar for band k
                return tabt[:, i * NB + k: i * NB + k + 1]

            def mm(accT, wi, pair, si, start, stop):
                sl = slice(si * CHUNK, (si + 1) * CHUNK)
                nc.tensor.matmul(accT[:, sl], wtile[:, 2 * wi:2 * wi + 2, :],
                                 pair[:, :, sl], start=start, stop=stop,
                                 perf_mode=dr)

            # staircase code = (x>=t1)+(x>=t2)+(x>=t3) as fp8 {0,1} masks
            # written into DoubleRow pair tiles:
            #   pab[:,0]=c1, pab[:,1]=c3 (both DVE, weights (Wj,Wj))
            #   pc[:,j%2]=c2              (weights (Wj,Wj+1))
            # Pool owns c2 by default (starts at band 0); C2ENG moves selected
            # pieces to DVE so neither engine backlogs into the drain tail.
            # Tail bands stream in pieces (SPLITS) so the final dependency
            # chain only covers a fraction of a band.
            acc = None
            pc = None
            ot = None
            out_specs = []
            for k in range(NB):
                j = k % 4
                g = k // 4
                last_group = g == NG - 1
                ck = tail_chunk if last_group else CHUNK
                if j == 0:
                    acc = ppool.tile([P, G], f32)
                    ot = opool.tile([P, G], u8)
                if j in (0, 2):
                    pc = pcpool.tile([P, 2, G], fp8, tag="pc")
                pab = pabpool.tile([P, 2, G], fp8, tag="pab")
                xt = xpool.tile([P, G], f32, tag="xt")

                npiece = splits.get(k, 1)
                pw = G // npiece
                engs = c2eng.get(k, "g")
                c2w = G // len(engs)
                # input DMA + c1/c3 (DVE) per input piece; c2 per c2-piece on
                # its assigned engine; then the band's matmuls/ACTs.  Issue
                # position only orders the queues — the dep tracker gates each
                # matmul on the actual mask writes covering its chunk.
                for pi in range(npiece):
                    hsl = slice(pi * pw, (pi + 1) * pw)
                    nc.sync.dma_start(out=xt[:, hsl],
                                      in_=x[:, k * G + hsl.start:k * G + hsl.stop])
                    nc.vector.tensor_scalar(pab[:, 0, hsl], xt[:, hsl],
                                            col(0, k), None, alu.is_ge)
                    for ci, ech in enumerate(engs):
                        csl = slice(ci * c2w, (ci + 1) * c2w)
                        if not (hsl.start < csl.stop <= hsl.stop):
                            continue   # c2 piece completes in another piece
                        e2 = nc.vector if ech == "v" else nc.gpsimd
                        e2.tensor_scalar(pc[:, j % 2, csl], xt[:, csl],
                                         col(1, k), None, alu.is_ge)
                    nc.vector.tensor_scalar(pab[:, 1, hsl], xt[:, hsl],
                                            col(2, k), None, alu.is_ge)

                # PE: fp8 DoubleRow matmuls accumulate base-4 packed digits;
                # chain per chunk, start on the group's first pair, stop on
                # its last (pc of digits 2|3); completed chunks then go
                # PSUM -> u8 codes (ACT) and later out to DRAM.
                for si in range(G // ck):
                    sl = slice(si * ck, (si + 1) * ck)
                    nc.tensor.matmul(acc[:, sl],
                                     wtile[:, 2 * j:2 * j + 2, :],
                                     pab[:, :, sl], start=(j == 0),
                                     stop=False, perf_mode=dr)
                    if j in (1, 3):
                        wi = 4 + j // 2
                        nc.tensor.matmul(acc[:, sl],
                                         wtile[:, 2 * wi:2 * wi + 2, :],
                                         pc[:, :, sl], start=False,
                                         stop=(j == 3), perf_mode=dr)
                    if j == 3:
                        nc.scalar.activation(
                            ot[:, sl], acc[:, sl],
                            mybir.ActivationFunctionType.Copy,
                            bias=0.0, scale=1.0,
                        )
                        out_specs.append((g, sl, ot))

            # All out-DMAs issue on SP AFTER every input issue: interleaving
            # them with the input stream would push the last input (and the
            # whole drain tail) ~1.1us later, while the DMA device is idle
            # during the tail anyway.
            for g, sl, ott in out_specs:
                nc.sync.dma_start(
                    out=y[:, g * G + sl.start:g * G + sl.stop],
                    in_=ott[:, sl])

    nc.compile()
    return nc


def _get_program():
    if "prog" not in _PROG_CACHE:
        _PROG_CACHE["prog"] = _build_program()
    return _PROG_CACHE["prog"]


# ---------------------------------------------------------------- entry point
def _prepare_in_maps(melspecs, centroids):
    thr, sv = _exact_tables(centroids)
    tab = _make_tab(thr)
    wts = _make_wts()
    mel = np.asarray(melspecs, dtype=np.float32)
    in_maps = []
    for c in range(NCORES):
        shard = mel[c * BSH:(c + 1) * BSH].reshape(TOK, C)
        xcm = np.ascontiguousarray(shard.T).reshape(P, ROW)
        in_maps.append({"x": xcm, "tab": tab, "wts": wts})
    return in_maps, sv


def _gather_out(results, sv):
    outs = []
    for c in range(NCORES):
        yp = np.asarray(results[c]["y"]).reshape(4, 32, NG, G)  # [j, p', g, col]
        codes = np.empty((32, 4, NG, 4, G), np.uint8)           # [p', i, g, j, col]
        for i in range(4):
            codes[:, i] = ((yp >> (2 * i)) & 3).transpose(1, 2, 0, 3)
        codes_cm = codes.reshape(C, TOK)   # channel-major flat = [80, 32768]
        vals = sv[np.arange(C)[:, None], codes_cm]
        outs.append(np.ascontiguousarray(vals.T).reshape(BSH, T, C))
    return np.concatenate(outs, axis=0)


def run(melspecs, centroids, trace=False, **kw):
    from concourse.bass_utils import run_bass_kernel_spmd

    prog = _get_program()
    in_maps, sv = _prepare_in_maps(melspecs, centroids)
    res = run_bass_kernel_spmd(prog, in_maps, list(range(NCORES)),
                               trace=trace, **kw)
    return _gather_out(res.results, sv), res


def kernel(melspecs, centroids):
    out, _ = run(melspecs, centroids, trace=False)
    return out
